# revision 11
# baseline (speedup 1.0000x reference)
"""Grouped GEMM (MoE expert-parallel) Trainium2 Bass kernel.

8 independent problems C_i = A_i @ B_i with A_i [M_i, 1024], B_i [1024, 1024],
M = [512, 1024, 2048, 4096, 1536, 768, 3072, 2560] (15616 rows total).

Strategy (SPMD across 8 NeuronCores, one identical program):
  - Row-shard the union of all A rows into 128-row blocks (122 real blocks).
  - Each core processes exactly 16 blocks grouped into 2 "B slots" of
    12/4 blocks; each slot is bound to one expert's B matrix. A
    hand-solved assignment maps (expert, block-range) pieces onto core
    slots with only 6 padding blocks of waste, so per-core DMA is
    8MB A + 8MB B + 8MB C.
  - The host pre-transposes each A block to [K, M] layout (K%128 on
    partitions) so the TensorEngine needs no on-chip transposes, and
    pre-permutes B to [p, kchunk, n] so all DMAs move 4KB contiguous
    rows per partition.
  - Matmuls run in float32r (hw rounds inputs to ~13-bit mantissa,
    1 cycle/row at moving free dim >= 256 vs 4 for plain fp32)
    accumulating over the 8 K-chunks in PSUM.
"""

import sys

for _p in ("/opt/trn_rl_repo",):
    if _p not in sys.path:
        sys.path.insert(0, _p)

import numpy as np

from concourse import bacc, mybir, tile
from concourse.bass_utils import run_bass_kernel_spmd

_MS = [512, 1024, 2048, 4096, 1536, 768, 3072, 2560]
_K = 1024
_N = 1024
_P = 128
_KC = _K // _P  # 8 K-chunks
_BLOCKS_PER_CORE = 16
_SLOT_SIZES = (12, 4)  # blocks per B slot
_SLOT_OF = [0] * 12 + [1] * 4


# Per core: 16 entries of (expert, block_idx, is_pad). Slot boundary at 12.
# Hand-solved so every slot's blocks come from a single expert and all 122
# real blocks are covered exactly once (6 pads total).
def _build_assignment():
    def span(e, lo, hi):
        return [(e, b, False) for b in range(lo, hi)]

    def pad(e):
        return [(e, 0, True)]  # duplicate expert-e block 0; output discarded

    assign = [
        span(3, 0, 12) + span(3, 24, 28),
        span(3, 12, 24) + span(3, 28, 32),
        span(6, 0, 12) + span(2, 12, 16),
        span(6, 12, 24) + span(7, 12, 16),
        span(2, 0, 12) + span(7, 16, 20),
        span(7, 0, 12) + span(5, 0, 4),
        span(4, 0, 12) + span(5, 4, 6) + pad(5) * 2,
        span(1, 0, 8) + pad(1) * 4 + span(0, 0, 4),
    ]
    slot_expert = [
        [3, 3],
        [3, 3],
        [6, 2],
        [6, 7],
        [2, 7],
        [7, 5],
        [4, 5],
        [1, 0],
    ]
    return assign, slot_expert


_ASSIGN, _SLOT_EXPERT = _build_assignment()

_PROGRAM = None


def _build_program():
    f32 = mybir.dt.float32
    f32r = mybir.dt.float32r

    nc = bacc.Bacc("TRN2", target_bir_lowering=False, debug=False, num_devices=8)

    at_d = nc.dram_tensor(
        "at_in", [_BLOCKS_PER_CORE, _P, _KC, _P], f32r, kind="ExternalInput"
    ).ap()
    b_d = nc.dram_tensor(
        "b_in", [len(_SLOT_SIZES), _P, _KC, _N], f32r, kind="ExternalInput"
    ).ap()
    out_d = nc.dram_tensor(
        "out", [_BLOCKS_PER_CORE, _P, _N], f32, kind="ExternalOutput"
    ).ap()

    with tile.TileContext(nc) as tc:
        with (
            tc.tile_pool(name="bpool", bufs=1) as bpool,
            tc.tile_pool(name="apool", bufs=6) as apool,
            tc.tile_pool(name="opool", bufs=4) as opool,
            tc.tile_pool(name="psum", bufs=4, space="PSUM") as psum_pool,
        ):
            b_sb = [
                bpool.tile([_P, _KC, _N], f32r, name=f"bslot{s}")
                for s in range(len(_SLOT_SIZES))
            ]

            # Prime the pipeline: block 0/1 A tiles first (SP queue), split
            # per K-chunk so the first matmul waits on one 64KB transfer;
            # slot-0 B chunks go on the Activation queue so A and B
            # transfers never head-of-line block each other.
            a_tiles = {}
            for j in (0, 1):
                a_tiles[j] = apool.tile([_P, _KC, _P], f32r, name="a_t")
                nc.sync.dma_start(a_tiles[j][:], at_d[j])
            for kc in range(_KC):
                nc.scalar.dma_start(b_sb[0][:, kc, :], b_d[0, :, kc, :])

            for j in range(_BLOCKS_PER_CORE):
                s = _SLOT_OF[j]
                if j in a_tiles:
                    a_sb = a_tiles.pop(j)
                else:
                    a_sb = apool.tile([_P, _KC, _P], f32r, name="a_t")
                    nc.sync.dma_start(a_sb[:], at_d[j])

                # Slot-1 B lands chunk-by-chunk while slot-0 still computes,
                # on the Activation queue so it never blocks A loads on SP.
                if 4 <= j < 4 + _KC:
                    kc = j - 4
                    nc.scalar.dma_start(b_sb[1][:, kc, :], b_d[1, :, kc, :])

                o_sb = opool.tile([_P, _N], f32, name="o_t")
                for nh in range(2):
                    ps = psum_pool.tile([_P, 512], f32, name="ps")
                    for kc in range(_KC):
                        nc.tensor.matmul(
                            ps[:],
                            a_sb[:, kc, :],
                            b_sb[s][:, kc, nh * 512 : (nh + 1) * 512],
                            start=(kc == 0),
                            stop=(kc == _KC - 1),
                        )
                    if nh == 0:
                        nc.scalar.copy(o_sb[:, :512], ps[:])
                    else:
                        nc.vector.tensor_copy(o_sb[:, 512:], ps[:])
                # GpSimd (otherwise idle) owns the store queue so a store
                # waiting on compute never head-of-line blocks A prefetches
                # on SP or B loads on Activation.
                nc.gpsimd.dma_start(out_d[j], o_sb[:])

    nc.compile()
    return nc


def _get_program():
    global _PROGRAM
    if _PROGRAM is None:
        _PROGRAM = _build_program()
    return _PROGRAM


def _make_in_maps(inputs):
    A = [np.ascontiguousarray(inputs[f"a{i}"], dtype=np.float32) for i in range(8)]
    B = [np.ascontiguousarray(inputs[f"b{i}"], dtype=np.float32) for i in range(8)]
    in_maps = []
    for c in range(8):
        at = np.empty((_BLOCKS_PER_CORE, _P, _KC, _P), np.float32)
        for j, (e, blk, _is_pad) in enumerate(_ASSIGN[c]):
            ab = A[e][blk * _P : (blk + 1) * _P]  # [128, 1024]
            at[j] = ab.reshape(_P, _KC, _P).transpose(2, 1, 0)  # [p, kc, m]
        bl = np.empty((len(_SLOT_SIZES), _P, _KC, _N), np.float32)
        for s in range(len(_SLOT_SIZES)):
            bl[s] = (
                B[_SLOT_EXPERT[c][s]].reshape(_KC, _P, _N).transpose(1, 0, 2)
            )  # [p, kc, n]
        in_maps.append({"at_in": at, "b_in": bl})
    return in_maps


def _scatter_outputs(results):
    outs = [np.empty((M, _N), np.float32) for M in _MS]
    for c in range(8):
        o = results[c]["out"]  # [16, 128, 1024]
        for j, (e, blk, is_pad) in enumerate(_ASSIGN[c]):
            if not is_pad:
                outs[e][blk * _P : (blk + 1) * _P] = o[j]
    return tuple(outs)


def kernel(**inputs):
    nc = _get_program()
    in_maps = _make_in_maps(inputs)
    res = run_bass_kernel_spmd(nc, in_maps, list(range(8)))
    return _scatter_outputs(res.results)


def run_traced(inputs, tmpdir=None):
    """test.py helper: same as kernel() but returns (outputs, BassKernelResults)
    with profiling enabled."""
    nc = _get_program()
    in_maps = _make_in_maps(inputs)
    res = run_bass_kernel_spmd(
        nc, in_maps, list(range(8)), trace=True, tmpdir=tmpdir
    )
    return _scatter_outputs(res.results), res


# revision 12
# speedup vs baseline: 1.0494x; 1.0494x over previous
"""Grouped GEMM (MoE expert-parallel) Trainium2 Bass kernel.

8 independent problems C_i = A_i @ B_i with A_i [M_i, 1024], B_i [1024, 1024],
M = [512, 1024, 2048, 4096, 1536, 768, 3072, 2560] (15616 rows total).

Strategy (SPMD across 8 NeuronCores, one identical program):
  - Row-shard the union of all A rows into 128-row blocks (122 real blocks).
  - Each core processes exactly 16 blocks grouped into 2 "B slots" of
    12/4 blocks; each slot is bound to one expert's B matrix. A
    hand-solved assignment maps (expert, block-range) pieces onto core
    slots with only 6 padding blocks of waste, so per-core DMA is
    8MB A + 8MB B + 8MB C.
  - The host pre-transposes each A block to [K, M] layout (K%128 on
    partitions) so the TensorEngine needs no on-chip transposes, and
    pre-permutes B to [p, kchunk, n] so all DMAs move 4KB contiguous
    rows per partition.
  - Matmuls run in float32r (hw rounds inputs to ~13-bit mantissa,
    1 cycle/row at moving free dim >= 256 vs 4 for plain fp32)
    accumulating over the 8 K-chunks in PSUM.
"""

import sys

for _p in ("/opt/trn_rl_repo",):
    if _p not in sys.path:
        sys.path.insert(0, _p)

import numpy as np

from concourse import bacc, mybir, tile
from concourse.bass_utils import run_bass_kernel_spmd

_MS = [512, 1024, 2048, 4096, 1536, 768, 3072, 2560]
_K = 1024
_N = 1024
_P = 128
_KC = _K // _P  # 8 K-chunks
_BLOCKS_PER_CORE = 16
_SLOT_SIZES = (12, 4)  # blocks per B slot
_SLOT_OF = [0] * 12 + [1] * 4


# Per core: 16 entries of (expert, block_idx, is_pad). Slot boundary at 12.
# Hand-solved so every slot's blocks come from a single expert and all 122
# real blocks are covered exactly once (6 pads total).
def _build_assignment():
    def span(e, lo, hi):
        return [(e, b, False) for b in range(lo, hi)]

    def pad(e):
        return [(e, 0, True)]  # duplicate expert-e block 0; output discarded

    assign = [
        span(3, 0, 12) + span(3, 24, 28),
        span(3, 12, 24) + span(3, 28, 32),
        span(6, 0, 12) + span(2, 12, 16),
        span(6, 12, 24) + span(7, 12, 16),
        span(2, 0, 12) + span(7, 16, 20),
        span(7, 0, 12) + span(5, 0, 4),
        span(4, 0, 12) + span(5, 4, 6) + pad(5) * 2,
        span(1, 0, 8) + pad(1) * 4 + span(0, 0, 4),
    ]
    slot_expert = [
        [3, 3],
        [3, 3],
        [6, 2],
        [6, 7],
        [2, 7],
        [7, 5],
        [4, 5],
        [1, 0],
    ]
    return assign, slot_expert


_ASSIGN, _SLOT_EXPERT = _build_assignment()

_PROGRAM = None


def _build_program():
    f32 = mybir.dt.float32
    f32r = mybir.dt.float32r

    nc = bacc.Bacc("TRN2", target_bir_lowering=False, debug=False, num_devices=8)

    at_d = nc.dram_tensor(
        "at_in", [_BLOCKS_PER_CORE, _P, _KC, _P], f32r, kind="ExternalInput"
    ).ap()
    b_d = nc.dram_tensor(
        "b_in", [len(_SLOT_SIZES), _P, _KC, _N], f32r, kind="ExternalInput"
    ).ap()
    out_d = nc.dram_tensor(
        "out", [_BLOCKS_PER_CORE, _P, _N], f32, kind="ExternalOutput"
    ).ap()

    with tile.TileContext(nc) as tc:
        with (
            tc.tile_pool(name="bpool", bufs=1) as bpool,
            tc.tile_pool(name="apool", bufs=4) as apool,
            tc.tile_pool(name="opool", bufs=4) as opool,
            tc.tile_pool(name="psum", bufs=4, space="PSUM") as psum_pool,
        ):
            b_sb = [
                bpool.tile([_P, _KC, _N], f32r, name=f"bslot{s}")
                for s in range(len(_SLOT_SIZES))
            ]

            # Prime the pipeline: block 0/1 A tiles first (SP queue), split
            # per K-chunk so the first matmul waits on one 64KB transfer;
            # slot-0 B chunks go on the Activation queue so A and B
            # transfers never head-of-line block each other.
            a_tiles = {}
            for j in (0, 1):
                a_tiles[j] = apool.tile([_P, _KC, _P], f32r, name="a_t")
                nc.sync.dma_start(a_tiles[j][:], at_d[j])
            for kc in range(_KC):
                nc.scalar.dma_start(b_sb[0][:, kc, :], b_d[0, :, kc, :])

            for j in range(_BLOCKS_PER_CORE):
                s = _SLOT_OF[j]
                if j in a_tiles:
                    a_sb = a_tiles.pop(j)
                else:
                    a_sb = apool.tile([_P, _KC, _P], f32r, name="a_t")
                    nc.sync.dma_start(a_sb[:], at_d[j])

                # Slot-1 B lands chunk-by-chunk while slot-0 still computes,
                # on the Activation queue so it never blocks A loads on SP.
                if 4 <= j < 4 + _KC:
                    kc = j - 4
                    nc.scalar.dma_start(b_sb[1][:, kc, :], b_d[1, :, kc, :])

                o_sb = opool.tile([_P, _N], f32, name="o_t")
                for nh in range(2):
                    ps = psum_pool.tile([_P, 512], f32, name="ps")
                    for kc in range(_KC):
                        nc.tensor.matmul(
                            ps[:],
                            a_sb[:, kc, :],
                            b_sb[s][:, kc, nh * 512 : (nh + 1) * 512],
                            start=(kc == 0),
                            stop=(kc == _KC - 1),
                        )
                    if nh == 0:
                        nc.scalar.copy(o_sb[:, :512], ps[:])
                        if j == _BLOCKS_PER_CORE - 1:
                            # tail block: flush the first half early so the
                            # final store only covers 256KB
                            nc.gpsimd.dma_start(out_d[j, :, :512], o_sb[:, :512])
                    else:
                        nc.vector.tensor_copy(o_sb[:, 512:], ps[:])
                # GpSimd (otherwise idle) owns the store queue so a store
                # waiting on compute never head-of-line blocks A prefetches
                # on SP or B loads on Activation.
                if j == _BLOCKS_PER_CORE - 1:
                    nc.gpsimd.dma_start(out_d[j, :, 512:], o_sb[:, 512:])
                else:
                    nc.gpsimd.dma_start(out_d[j], o_sb[:])

    nc.compile()
    return nc


def _get_program():
    global _PROGRAM
    if _PROGRAM is None:
        _PROGRAM = _build_program()
    return _PROGRAM


def _make_in_maps(inputs):
    A = [np.ascontiguousarray(inputs[f"a{i}"], dtype=np.float32) for i in range(8)]
    B = [np.ascontiguousarray(inputs[f"b{i}"], dtype=np.float32) for i in range(8)]
    in_maps = []
    for c in range(8):
        at = np.empty((_BLOCKS_PER_CORE, _P, _KC, _P), np.float32)
        for j, (e, blk, _is_pad) in enumerate(_ASSIGN[c]):
            ab = A[e][blk * _P : (blk + 1) * _P]  # [128, 1024]
            at[j] = ab.reshape(_P, _KC, _P).transpose(2, 1, 0)  # [p, kc, m]
        bl = np.empty((len(_SLOT_SIZES), _P, _KC, _N), np.float32)
        for s in range(len(_SLOT_SIZES)):
            bl[s] = (
                B[_SLOT_EXPERT[c][s]].reshape(_KC, _P, _N).transpose(1, 0, 2)
            )  # [p, kc, n]
        in_maps.append({"at_in": at, "b_in": bl})
    return in_maps


def _scatter_outputs(results):
    outs = [np.empty((M, _N), np.float32) for M in _MS]
    for c in range(8):
        o = results[c]["out"]  # [16, 128, 1024]
        for j, (e, blk, is_pad) in enumerate(_ASSIGN[c]):
            if not is_pad:
                outs[e][blk * _P : (blk + 1) * _P] = o[j]
    return tuple(outs)


def kernel(**inputs):
    nc = _get_program()
    in_maps = _make_in_maps(inputs)
    res = run_bass_kernel_spmd(nc, in_maps, list(range(8)))
    return _scatter_outputs(res.results)


def run_traced(inputs, tmpdir=None):
    """test.py helper: same as kernel() but returns (outputs, BassKernelResults)
    with profiling enabled."""
    nc = _get_program()
    in_maps = _make_in_maps(inputs)
    res = run_bass_kernel_spmd(
        nc, in_maps, list(range(8)), trace=True, tmpdir=tmpdir
    )
    return _scatter_outputs(res.results), res
